# revision 8
# baseline (speedup 1.0000x reference)
"""MiniCPM attention (GQA + RoPE + causal softmax + o_proj) on 8 TRN2 NeuronCores.

Sharding: 8-way tensor parallel over heads for QKV-proj + attention
(core c owns q-heads 4c..4c+3 and kv-head c, both batches), then one
8-rank AllToAll re-shards activations from head-split to token-split,
and each core runs o_proj (full contraction) for its 512-token block.

Layout is feature-major ("transposed"): activations live as [feat, tok]
so matmuls run directly off the projection outputs, softmax runs on
scores^T tiles [k_pos, q_pos] (sums over k via a ones-column folded
into the PV matmul), and no transposes of the big probability matrix
are ever needed.
"""

import sys

for _p in ("/opt/trn_rl_repo", "/root/.axon_site/_ro/trn_rl_repo"):
    if _p not in sys.path:
        sys.path.insert(0, _p)

import numpy as np
import ml_dtypes

import concourse.bacc as bacc
import concourse.mybir as mybir
import concourse.tile as tile
from concourse.bass import ts
from concourse.bass_utils import run_bass_kernel_spmd

BF16 = ml_dtypes.bfloat16

H = 2048
NH = 32
NKV = 8
HD = 64
B = 2
S = 2048
NC = 8           # cores
HL = NH // NC    # q heads per core = 4
TOK = B * S      # 4096
NT = TOK // 512  # 8 tok chunks of 512
KT = S // 128    # 16 k tiles per batch
QC = S // 512    # 4 q chunks per batch

_cache = {}


def _build(causal: bool):
    nc = bacc.Bacc("TRN2", target_bir_lowering=False, debug=False,
                   num_devices=NC, enable_asserts=False)
    f32 = mybir.dt.float32
    bf16 = mybir.dt.bfloat16

    xT = nc.dram_tensor("xT", [H, TOK], bf16, kind="ExternalInput").ap()
    wq = nc.dram_tensor("wq", [H, HL * HD], bf16, kind="ExternalInput").ap()
    wkv = nc.dram_tensor("wkv", [H, 2 * HD], bf16, kind="ExternalInput").ap()
    wo = nc.dram_tensor("wo", [H, H], bf16, kind="ExternalInput").ap()
    cos_q = nc.dram_tensor("cos_q", [HD, S], f32, kind="ExternalInput").ap()
    sin_q = nc.dram_tensor("sin_q", [HD, S], f32, kind="ExternalInput").ap()
    cos_k = nc.dram_tensor("cos_k", [HD, S], f32, kind="ExternalInput").ap()
    sin_k = nc.dram_tensor("sin_k", [HD, S], f32, kind="ExternalInput").ap()

    o_t = nc.dram_tensor("o_t", [H, 512], f32, kind="ExternalOutput").ap()
    k_t = nc.dram_tensor("k_t", [HD, TOK], f32, kind="ExternalOutput").ap()
    v_t = nc.dram_tensor("v_t", [HD, TOK], f32, kind="ExternalOutput").ap()

    with tile.TileContext(nc) as tc:
        with (
            tc.tile_pool(name="weights", bufs=1) as wpool,
            tc.tile_pool(name="persist", bufs=1) as perm,
            tc.tile_pool(name="xin", bufs=2) as xin,
            tc.tile_pool(name="rope_tmp", bufs=2) as rtmp,
            tc.tile_pool(name="small", bufs=4) as small,
            tc.tile_pool(name="probs", bufs=4) as probs_pool,
            tc.tile_pool(name="psA", bufs=4, space="PSUM") as psA,
            tc.tile_pool(name="psB", bufs=4, space="PSUM") as psB,
            tc.tile_pool(name="oproj", bufs=4) as opool,
            tc.tile_pool(name="dram", bufs=1, space="DRAM") as dram,
        ):
            # ---- resident weights ----
            wq_sb = wpool.tile([128, KT, HL * HD], bf16)
            nc.sync.dma_start(wq_sb[:], wq.rearrange("(k p) m -> p k m", p=128))
            wkv_sb = wpool.tile([128, KT, 2 * HD], bf16)
            nc.sync.dma_start(wkv_sb[:], wkv.rearrange("(k p) m -> p k m", p=128))
            cq = wpool.tile([HD, S], f32)
            nc.sync.dma_start(cq[:], cos_q[:])
            sq = wpool.tile([HD, S], f32)
            nc.sync.dma_start(sq[:], sin_q[:])
            ck = wpool.tile([HD, S], f32)
            nc.sync.dma_start(ck[:], cos_k[:])
            sk = wpool.tile([HD, S], f32)
            nc.sync.dma_start(sk[:], sin_k[:])
            ident = wpool.tile([128, 128], bf16)
            from concourse.masks import make_identity
            make_identity(nc, ident[:])

            # ---- persistent activations ----
            qbf = [perm.tile([HD, TOK], bf16, tag=f"qbf{h}", name=f"qbf{h}") for h in range(HL)]
            kbf = perm.tile([HD, TOK], bf16)
            # v (natural layout) + ones column per 128-token tile: [128, 65] slabs
            vno = perm.tile([128, (TOK // 128) * (HD + 1)], bf16)
            nc.vector.memset(vno[:], 1.0)
            # attention output (normalized), feature-major: 2 tiles of 2 heads
            ao = [perm.tile([128, TOK], bf16, tag=f"ao{i}", name=f"ao{i}") for i in range(2)]

            # ================= QKV projection + RoPE =================
            for t in range(NT):
                pos = (t * 512) % S  # position within batch for rope tables
                x_sb = xin.tile([128, KT, 512], bf16)
                nc.sync.dma_start(
                    x_sb[:], xT[:, ts(t, 512)].rearrange("(k p) n -> p k n", p=128)
                )
                ps_q = [psA.tile([128, 512], f32, tag="psA", name=f"psq{m}") for m in range(2)]
                ps_kv = psA.tile([128, 512], f32, tag="psA")
                for m in range(2):
                    for k in range(KT):
                        nc.tensor.matmul(
                            ps_q[m][:], wq_sb[:, k, ts(m, 128)], x_sb[:, k, :],
                            start=(k == 0), stop=(k == KT - 1),
                        )
                for k in range(KT):
                    nc.tensor.matmul(
                        ps_kv[:], wkv_sb[:, k, :], x_sb[:, k, :],
                        start=(k == 0), stop=(k == KT - 1),
                    )

                # ---- RoPE on the 4 q heads of this chunk ----
                for h in range(HL):
                    src = ps_q[h // 2][(h % 2) * HD:(h % 2) * HD + HD, :]
                    qpre = rtmp.tile([HD, 512], f32, tag="qpre")
                    nc.scalar.copy(qpre[:], src)
                    tmp = rtmp.tile([HD, 512], f32, tag="tmp")
                    nc.vector.tensor_copy(tmp[0:32, :], qpre[32:64, :])
                    nc.vector.tensor_copy(tmp[32:64, :], qpre[0:32, :])
                    t1 = rtmp.tile([HD, 512], f32, tag="t1")
                    nc.vector.tensor_mul(t1[:], qpre[:], cq[:, ts(pos // 512, 512)])
                    t2 = rtmp.tile([HD, 512], f32, tag="t2")
                    nc.vector.tensor_mul(t2[:], tmp[:], sq[:, ts(pos // 512, 512)])
                    nc.vector.tensor_add(qbf[h][:, ts(t, 512)], t1[:], t2[:])

                # ---- RoPE on k; fp32 k goes straight out ----
                kpre = rtmp.tile([HD, 512], f32, tag="qpre")
                nc.scalar.copy(kpre[:], ps_kv[0:HD, :])
                tmp = rtmp.tile([HD, 512], f32, tag="tmp")
                nc.vector.tensor_copy(tmp[0:32, :], kpre[32:64, :])
                nc.vector.tensor_copy(tmp[32:64, :], kpre[0:32, :])
                t1 = rtmp.tile([HD, 512], f32, tag="t1")
                nc.vector.tensor_mul(t1[:], kpre[:], ck[:, ts(pos // 512, 512)])
                t2 = rtmp.tile([HD, 512], f32, tag="t2")
                nc.vector.tensor_mul(t2[:], tmp[:], sk[:, ts(pos // 512, 512)])
                krot = rtmp.tile([HD, 512], f32, tag="krot")
                nc.vector.tensor_add(krot[:], t1[:], t2[:])
                nc.sync.dma_start(k_t[:, ts(t, 512)], krot[:])
                nc.vector.tensor_copy(kbf[:, ts(t, 512)], krot[:])

                # ---- v: fp32 out + bf16 transpose into natural layout ----
                vf = rtmp.tile([HD, 512], f32, tag="vf")
                nc.scalar.copy(vf[:], ps_kv[HD:128, :])
                nc.sync.dma_start(v_t[:, ts(t, 512)], vf[:])
                vb = rtmp.tile([HD, 512], bf16, tag="vb")
                nc.vector.tensor_copy(vb[:], ps_kv[HD:128, :])
                for i in range(4):  # 4 x 128-token sub-tiles
                    pt = psA.tile([128, HD], bf16, tag="psA", name="vtp")
                    nc.tensor.transpose(pt[:], vb[:, ts(i, 128)], ident[0:HD, 0:HD])
                    g = t * 4 + i  # global 128-token tile index
                    nc.vector.tensor_copy(vno[:, g * (HD + 1):g * (HD + 1) + HD], pt[:])

            # ================= attention =================
            for b in range(B):
                for h in range(HL):
                    pv = [psB.tile([128, 512], f32, tag="psB", name=f"pv{q}") for q in range(QC)]
                    for kt in range(KT):
                        qc0 = (kt // 4) if causal else 0
                        for qc in range(qc0, QC):
                            sc = psA.tile([128, 512], f32, tag="psA")
                            nc.tensor.matmul(
                                sc[:],
                                kbf[:, b * S + kt * 128:b * S + kt * 128 + 128],
                                qbf[h][:, b * S + qc * 512:b * S + qc * 512 + 512],
                                start=True, stop=True,
                            )
                            pt = probs_pool.tile([128, 512], bf16, tag="pt")
                            nc.scalar.activation(
                                pt[:], sc[:], mybir.ActivationFunctionType.Exp
                            )
                            if causal and kt >= 4 * qc:
                                # keep where (kt*128 + p) <= (qc*512 + j)
                                nc.gpsimd.affine_select(
                                    out=pt[:], in_=pt[:],
                                    pattern=[[1, 512]],
                                    compare_op=mybir.AluOpType.is_ge,
                                    fill=0.0,
                                    base=qc * 512 - kt * 128,
                                    channel_multiplier=-1,
                                )
                            g = (b * S // 128) + kt
                            last = min(KT - 1, 4 * qc + 3) if causal else KT - 1
                            nc.tensor.matmul(
                                pv[qc][0:HD + 1, :],
                                vno[:, g * (HD + 1):(g + 1) * (HD + 1)],
                                pt[:],
                                start=(kt == 0), stop=(kt == last),
                            )
                    for qc in range(QC):
                        rl = small.tile([1, 512], f32, tag="rl")
                        nc.vector.reciprocal(rl[:], pv[qc][HD:HD + 1, :])
                        bc = small.tile([HD, 512], f32, tag="bc")
                        nc.gpsimd.partition_broadcast(bc[:], rl[:])
                        dst = ao[h // 2][
                            (h % 2) * HD:(h % 2) * HD + HD,
                            b * S + qc * 512:b * S + qc * 512 + 512,
                        ]
                        nc.vector.tensor_mul(dst, pv[qc][0:HD, :], bc[:])

            # ================= AllToAll: head-split -> token-split =================
            a2a_in = dram.tile([NC * 256 * 512], bf16)
            a2a_out = dram.tile([NC * 256 * 512], bf16)
            # partition dim must stay first on the SBUF side of the DMA, so
            # give the DRAM side a feature-major view [f, s, n]
            a2a_in_f = a2a_in.rearrange("(s f n) -> f s n", s=NC, f=256)
            a2a_out_v = a2a_out.rearrange("(s f n) -> s f n", s=NC, f=256)
            for i in range(2):
                nc.sync.dma_start(
                    a2a_in_f[ts(i, 128), :, :],
                    ao[i][:].rearrange("p (s n) -> p s n", s=NC),
                )
            nc.gpsimd.collective_compute(
                "AllToAll", mybir.AluOpType.bypass,
                replica_groups=[list(range(NC))],
                ins=[a2a_in.opt()], outs=[a2a_out.opt()],
            )

            # ================= o_proj for this core's 512 tokens =================
            aoall = [opool.tile([128, 512], bf16, tag=f"aoall{m}", bufs=1,
                                  name=f"aoall{m}") for m in range(16)]
            for m in range(16):
                nc.sync.dma_start(
                    aoall[m][:],
                    a2a_out_v[(m * 128) // 256, (m * 128) % 256:(m * 128) % 256 + 128, :],
                )
            for m in range(16):
                po = psA.tile([128, 512], f32, tag="psA")
                for k in range(16):
                    wo_sb = opool.tile([128, 128], bf16, tag="wo")
                    nc.sync.dma_start(
                        wo_sb[:], wo[ts(k, 128), ts(m, 128)]
                    )
                    nc.tensor.matmul(
                        po[:], wo_sb[:], aoall[k][:],
                        start=(k == 0), stop=(k == 15),
                    )
                ot = opool.tile([128, 512], f32, tag="ot")
                nc.scalar.copy(ot[:], po[:])
                nc.sync.dma_start(o_t[ts(m, 128), :], ot[:])

    nc.compile()
    return nc


def _get(causal: bool):
    if causal not in _cache:
        _cache[causal] = _build(causal)
    return _cache[causal]


def _host_inputs(hidden_states, cos, sin, Wq, Wk, Wv, Wo):
    x = np.asarray(hidden_states, np.float32).reshape(TOK, H)
    xT = np.ascontiguousarray(x.T).astype(BF16)
    cosT = np.ascontiguousarray(np.asarray(cos, np.float32).T)  # [HD, S]
    sinT = np.ascontiguousarray(np.asarray(sin, np.float32).T)
    sgn = np.concatenate([-np.ones((HD // 2, 1), np.float32),
                          np.ones((HD // 2, 1), np.float32)], axis=0)
    # rotate-half becomes a 32-partition swap; the sign lives in the sin table
    sinT_s = sinT * sgn
    scale = 1.0 / np.sqrt(HD)
    common = {
        "xT": xT,
        "wo": np.asarray(Wo, np.float32).astype(BF16),
        "cos_q": (cosT * scale).astype(np.float32),
        "sin_q": (sinT_s * scale).astype(np.float32),
        "cos_k": cosT.astype(np.float32),
        "sin_k": sinT_s.astype(np.float32),
    }
    Wq = np.asarray(Wq, np.float32)
    Wk = np.asarray(Wk, np.float32)
    Wv = np.asarray(Wv, np.float32)
    maps = []
    for c in range(NC):
        m = dict(common)
        m["wq"] = np.ascontiguousarray(Wq[:, c * HL * HD:(c + 1) * HL * HD]).astype(BF16)
        m["wkv"] = np.concatenate(
            [Wk[:, c * HD:(c + 1) * HD], Wv[:, c * HD:(c + 1) * HD]], axis=1
        ).astype(BF16)
        maps.append(m)
    return maps


def kernel(hidden_states, cos, sin, Wq, Wk, Wv, Wo, is_causal):
    causal = bool(int(np.asarray(is_causal)))
    nc = _get(causal)
    in_maps = _host_inputs(hidden_states, cos, sin, Wq, Wk, Wv, Wo)
    res = run_bass_kernel_spmd(nc, in_maps, list(range(NC))).results

    attn = np.empty((TOK, H), np.float32)
    k_out = np.empty((B, NKV, S, HD), np.float32)
    v_out = np.empty((B, NKV, S, HD), np.float32)
    for c in range(NC):
        attn[c * 512:(c + 1) * 512, :] = res[c]["o_t"].T
        kc = res[c]["k_t"].reshape(HD, B, S)
        vc = res[c]["v_t"].reshape(HD, B, S)
        k_out[:, c, :, :] = kc.transpose(1, 2, 0)
        v_out[:, c, :, :] = vc.transpose(1, 2, 0)
    return attn.reshape(B, S, H), (k_out, v_out)
